# revision 11
# baseline (speedup 1.0000x reference)
"""DeepSet-equivariant layer on 8 TRN2 NeuronCores.

Math (reference):
    y = x @ w1 + (colsum(x) @ w2) / n + bias        x: (n, 128)

Distribution (sharding_hint): shard x and y along the set dimension n
across the 8 cores; each core computes its local column-sum, the cores
exchange the 128-float partial sums, and w1/w2/bias are replicated.

Two structural choices vs a straightforward port:

1. fp16 streaming.  The host pre-transposes each shard to [128, r] AND
   casts it to fp16 before upload; the device matmuls fp16 x fp16 with
   fp32 PSUM accumulation and writes the output back as fp16 (host casts
   to fp32).  This halves both HBM streams (the kernel is memory-bound),
   at ~1e-3 relative error (fp16 mult is exact to fp22, accum is fp32).

2. No ncfw collectives.  The 512-byte cross-core sum is done with SWDGE
   remote_dma_broadcast into XOR-indexed SBUF mailboxes: sender S's
   broadcast #d lands in mailbox column d of core S^d, so receiver R's
   column d holds the partial of core R^d -- each column has a unique
   sender and the global colsum is a free-axis reduce of the [128, 8]
   mailbox.  Arrival is signalled by the sending DMA bumping the
   receiver's semaphore (+2 per broadcast, 14 total).  Descriptor
   generation (7 Q7 preps) is issued at kernel start and hidden under
   the input stream; the colsum read is deferred to the single
   trigger_dma once the local partial is ready.  This replaces the
   warm-up + AllReduce pair (ncfw spin-up barrier + 2x ~10 us mesh
   execs, ~35-75 us on the critical path) with a ~2-4 us exchange.

Schedule: phase 1 streams all of xT into a resident SBUF buffer (DMA
bound; per-chunk column-sum reduces hide under the DMA), the exchange
fires, then phase 2 matmuls from resident x, fuses the +transmit+bias
add into the PSUM drain (alternating DVE/ACT), and streams fp16 results
out with 1 MiB coalesced DMAs.
"""

import numpy as np

import concourse.bass as bass
import concourse.tile as tile
from concourse import bacc, mybir
from concourse.bass_utils import run_bass_kernel_spmd

N_CORES = 8
D = 128                 # d_in == d_out
N_ROWS = 200000         # full set size
R = 25088               # padded rows per core: 8 * 25088 = 200704 >= 200000
IN_CHUNK = 4096         # columns per input DMA chunk (1 MiB fp16)
DRAIN_CHUNK = 2048      # columns per PSUM tile / fused-add chunk (4 banks)
OUT_GROUP = 4096        # columns per output DMA (1 MiB fp16)
MM_N = 512              # moving-operand free dim per matmul (1 PSUM bank)

F16 = mybir.dt.float16
F32 = mybir.dt.float32


def _split(r, step):
    out = []
    c0 = 0
    while c0 < r:
        cw = min(step, r - c0)
        out.append((c0, cw))
        c0 += cw
    return out


def build_nc(r: int, n_total: int):
    """Build the SPMD Bass program for one core holding r rows."""
    in_chunks = _split(r, IN_CHUNK)
    drain_chunks = _split(r, DRAIN_CHUNK)

    nc = bacc.Bacc(
        "TRN2",
        target_bir_lowering=False,
        debug=False,
        num_devices=N_CORES,
    )

    xt = nc.declare_dram_parameter("xt", [D, r], F16, isOutput=False)
    w1 = nc.declare_dram_parameter("w1", [D, D], F16, isOutput=False)
    w2 = nc.declare_dram_parameter("w2", [D, D], F32, isOutput=False)
    bias_c = nc.declare_dram_parameter("bias_c", [D, 1], F32, isOutput=False)
    out = nc.declare_dram_parameter("out", [D, r], F16, isOutput=True)

    # Cross-core exchange semaphores.  Same program on every core =>
    # identical indices, as remote_dma requires.
    rsem = nc.alloc_semaphore("cs_arrive")   # bumped by peers' DMAs
    lsem = nc.alloc_semaphore("cs_sent")     # local send completion

    # Fire-and-forget warm-up AllReduce, emitted BEFORE the TileContext.
    # Its result is unused; it exists because (a) a collective in the
    # NEFF marks the 8 cores as a gang (number_of_cc_participants=8) so
    # the runtime launches them together -- without it the cores start
    # ms apart and the colsum exchange inherits that stagger -- and
    # (b) its barrier absorbs ncfw wake + residual launch skew
    # concurrently with the input stream.
    ccw_in = nc.dram_tensor("ccw_in", [D, 1], F32)
    ccw_out = nc.dram_tensor("ccw_out", [D, 1], F32, addr_space="Shared")
    warm_sem = nc.alloc_semaphore("warm_cc")
    nc.gpsimd.collective_compute(
        "AllReduce",
        mybir.AluOpType.add,
        replica_groups=[list(range(N_CORES))],
        ins=[ccw_in.ap().opt()],
        outs=[ccw_out.ap().opt()],
    ).then_inc(warm_sem)

    with tile.TileContext(nc) as tc:
        with (
            tc.tile_pool(name="const", bufs=1) as const_pool,
            tc.tile_pool(name="xres", bufs=1) as xres_pool,
            tc.tile_pool(name="obuf", bufs=3) as obuf_pool,
            tc.tile_pool(name="small", bufs=1) as small_pool,
            tc.tile_pool(name="mm", bufs=2, space=bass.MemorySpace.PSUM) as mm_pool,
        ):
            w1_sb = const_pool.tile([D, D], F16)
            w2_sb = const_pool.tile([D, D], F32)
            bias_sb = const_pool.tile([D, 1], F32)

            x_sb = xres_pool.tile([D, r], F16)
            n_in = len(in_chunks)
            # Two colsum partials per full chunk (DVE half + ACT half) so
            # the reduces keep pace with the fp16 stream; tail chunk gets
            # a single DVE reduce.
            cs_parts = small_pool.tile([D, 2 * n_in], F32)
            trash = small_pool.tile([D, IN_CHUNK // 2], F16)
            # mbox col 0 = local partial; cols 1..7 filled by peers;
            # col 8 = zero, written by the arrival gate (see below).
            mbox = small_pool.tile([D, N_CORES + 1], F32)

            # Input chunk 0 first (sync ring), weights on the ACT ring.
            c0, cw = in_chunks[0]
            nc.sync.dma_start(x_sb[:, c0 : c0 + cw], xt[:, c0 : c0 + cw])
            nc.scalar.dma_start(w1_sb[:], w1[:, :])
            nc.scalar.dma_start(w2_sb[:], w2[:, :])
            nc.scalar.dma_start(bias_sb[:], bias_c[:, :])

            # Exchange descriptor preps: Q7 writes the descriptor rings
            # now (hidden under the stream); the mbox[:,0] read happens
            # at trigger time.  Broadcast #d -> peer tpb XOR d, mailbox
            # column d.  Slot d of rdests keeps cross-die dests (bit 2)
            # on D2D-capable lanes automatically.
            for d in range(1, N_CORES):
                rdests = [None] * N_CORES
                rdests[d] = (0, d)
                nc.gpsimd.remote_dma_broadcast(
                    mbox[:, d : d + 1],
                    mbox[:, 0:1],
                    rsem,
                    lsem,
                    rdests=rdests,
                )

            def emit_mms(ps, c0, cw):
                s0 = 0
                while s0 < cw:
                    sw = min(MM_N, cw - s0)
                    nc.tensor.matmul(
                        ps[:, s0 : s0 + sw],
                        w1_sb[:],
                        x_sb[:, c0 + s0 : c0 + s0 + sw],
                    )
                    s0 += sw

            # Phase 1: stream the rest of xT.  Column sums are split per
            # chunk across DVE (reduce) and ACT (activation w/ fp32
            # accumulator) halves so they keep pace with the stream.
            # Redundant matmul rounds into ps0 keep the PE's HAM clock
            # gate open during the stream, so phase 2 starts at full
            # clock instead of ramping from half rate.
            ps0 = mm_pool.tile([D, DRAIN_CHUNK], F32, tag="ps")
            for c, (c0, cw) in enumerate(in_chunks):
                if c > 0:
                    dma_eng = nc.sync if c % 2 == 0 else nc.scalar
                    dma_eng.dma_start(x_sb[:, c0 : c0 + cw], xt[:, c0 : c0 + cw])
                if cw == IN_CHUNK:
                    h = cw // 2
                    nc.vector.reduce_sum(
                        cs_parts[:, 2 * c : 2 * c + 1],
                        x_sb[:, c0 : c0 + h],
                        axis=mybir.AxisListType.X,
                    )
                    nc.scalar.activation(
                        trash[:, :h],
                        x_sb[:, c0 + h : c0 + cw],
                        mybir.ActivationFunctionType.Copy,
                        accum_out=cs_parts[:, 2 * c + 1 : 2 * c + 2],
                    )
                else:
                    nc.vector.reduce_sum(
                        cs_parts[:, 2 * c : 2 * c + 1],
                        x_sb[:, c0 : c0 + cw],
                        axis=mybir.AxisListType.X,
                    )
                    nc.vector.memset(cs_parts[:, 2 * c + 1 : 2 * c + 2], 0.0)
                if 0 < c < n_in - 1:
                    # PE warm-up: two redundant rounds over this chunk's
                    # head; all target ps0 (WAW-chained, overwritten by
                    # the real pre-fill below).
                    emit_mms(ps0, c0, DRAIN_CHUNK)
                    emit_mms(ps0, c0, DRAIN_CHUNK)

            # Local partial -> mbox col 0, then fire the exchange.
            nc.vector.reduce_sum(
                mbox[:, 0:1], cs_parts[:], axis=mybir.AxisListType.X
            )
            nc.gpsimd.trigger_dma(count=None)

            # Phase 2a (emitted before the gate => runs during the
            # exchange wait): matmul every chunk from resident x and
            # drain PSUM -> y0 as fp16 WITHOUT the transmit term
            # (alternating DVE/ACT).  These depend only on local data,
            # so on cores that wait for slow peers this work is free.
            y0 = xres_pool.tile([D, r], F16)
            for i, (c0, cw) in enumerate(drain_chunks):
                if i == 0:
                    ps = ps0
                else:
                    ps = mm_pool.tile([D, DRAIN_CHUNK], F32, tag="ps")
                emit_mms(ps, c0, cw)
                if i % 2 == 0:
                    nc.vector.tensor_scalar(
                        out=y0[:, c0 : c0 + cw],
                        in0=ps[:, :cw],
                        scalar1=1.0,
                        scalar2=None,
                        op0=mybir.AluOpType.mult,
                    )
                else:
                    nc.scalar.activation(
                        y0[:, c0 : c0 + cw],
                        ps[:, :cw],
                        mybir.ActivationFunctionType.Identity,
                        scale=1.0,
                    )

            # Arrival gate: writes mbox col 8 = col0 * 0.  Both edges are
            # same-engine DVE (RAW from the combine via col 0, RAW into
            # the mailbox reduce via col 8), so Tile orders
            # combine -> gate -> reduce without consuming the gate's HW
            # wait slot.  The actual rsem wait is attached to this
            # instruction AFTER scheduling (Tile's single-core scheduling
            # sim cannot model remotely-incremented semaphores and would
            # report a deadlock).
            gate = nc.vector.tensor_scalar(
                out=mbox[:, N_CORES : N_CORES + 1],
                in0=mbox[:, 0:1],
                scalar1=0.0,
                scalar2=None,
                op0=mybir.AluOpType.mult,
            )
            gcs = small_pool.tile([D, 1], F32)
            nc.vector.reduce_sum(gcs[:], mbox[:], axis=mybir.AxisListType.X)

            t_ps = mm_pool.tile([D, DRAIN_CHUNK], F32, tag="ps")
            nc.tensor.matmul(t_ps[:, :1], w2_sb[:], gcs[:])
            t_sb = small_pool.tile([D, 1], F32)
            nc.vector.tensor_scalar(
                out=t_sb[:],
                in0=t_ps[:, :1],
                scalar1=1.0 / float(n_total),
                scalar2=bias_sb[:],
                op0=mybir.AluOpType.mult,
                op1=mybir.AluOpType.add,
            )

            # Phase 2b (post-gate): in-place y0 += t in DVE/ACT halves,
            # then stream each 1 MiB group out.  Pure add+DMA -- all
            # matmul and conversion work already happened in 2a.
            n_dma = 0
            for g0, gw in _split(r, OUT_GROUP):
                h = gw // 2 if gw > DRAIN_CHUNK else gw
                nc.vector.tensor_scalar(
                    out=y0[:, g0 : g0 + h],
                    in0=y0[:, g0 : g0 + h],
                    scalar1=t_sb[:],
                    scalar2=None,
                    op0=mybir.AluOpType.add,
                )
                if gw > h:
                    nc.scalar.activation(
                        y0[:, g0 + h : g0 + gw],
                        y0[:, g0 + h : g0 + gw],
                        mybir.ActivationFunctionType.Identity,
                        bias=t_sb[:],
                        scale=1.0,
                    )
                dma_eng = nc.sync if n_dma % 2 == 0 else nc.scalar
                dma_eng.dma_start(out[:, g0 : g0 + gw], y0[:, g0 : g0 + gw])
                n_dma += 1

    # Attach the cross-core arrival wait post-scheduling: every peer's
    # broadcast bumps rsem by 2 on landing; 7 peers => 14.  check=False
    # because the gate already carries Tile's same-engine tick wait --
    # wait_op appends a second wait condition.  (No local wait on lsem
    # is needed: a core can only finish once its own rsem hits 14,
    # which requires every send in the system to have landed.)
    gate.wait_op(rsem, 2 * (N_CORES - 1), "sem-ge", check=False)

    nc.compile()
    return nc


_nc_cache: dict = {}


def _get_nc(r: int, n_total: int):
    key = (r, n_total)
    if key not in _nc_cache:
        _nc_cache[key] = build_nc(r, n_total)
    return _nc_cache[key]


LAST_RESULTS = None


def _execute(x, w1, w2, bias, r, trace=False, tmpdir=None, trace_cores=None):
    global LAST_RESULTS
    x = np.asarray(x, dtype=np.float32)
    w1 = np.asarray(w1, dtype=np.float32)
    w2 = np.ascontiguousarray(np.asarray(w2, dtype=np.float32))
    bias = np.asarray(bias, dtype=np.float32)
    n, d = x.shape
    assert d == D and r * N_CORES >= n

    xp = np.zeros((N_CORES * r, d), dtype=np.float16)
    xp[:n] = x.astype(np.float16)
    # (8, r, d) -> (8, d, r) pre-transposed fp16 shards
    xts = np.ascontiguousarray(xp.reshape(N_CORES, r, d).transpose(0, 2, 1))
    w1_h = np.ascontiguousarray(w1.astype(np.float16))
    bias_col = np.ascontiguousarray(bias.reshape(1, d).T)

    in_maps = [
        {"xt": xts[i], "w1": w1_h, "w2": w2, "bias_c": bias_col}
        for i in range(N_CORES)
    ]

    nc = _get_nc(r, n)
    kwargs = {}
    if trace:
        kwargs.update(trace=True, tmpdir=tmpdir)
        if trace_cores is not None:
            kwargs.update(trace_cores=trace_cores)
    res = run_bass_kernel_spmd(nc, in_maps, core_ids=list(range(N_CORES)), **kwargs)
    LAST_RESULTS = res

    yts = [res.results[i]["out"] for i in range(N_CORES)]  # each (D, r) fp16
    y = np.concatenate(
        [np.asarray(yt, dtype=np.float32).T for yt in yts], axis=0
    )[:n]
    return np.ascontiguousarray(y)


def kernel(x, w1, w2, bias):
    return _execute(x, w1, w2, bias, R)
